# revision 3
# baseline (speedup 1.0000x reference)
"""Trainium2 Bass kernel for the global-context-fusion block.

Reference computation (per batch sample b):
    pooled[c] = mean_{h,w} x[b,c,h,w]                         # [C]
    y1 = relu6(w_guide @ pooled)                              # [R]
    y2 = relu6((w_fuse @ y1 - bn_mean) * inv_std * g + beta)  # [C]
    out[b,c,h,w] = x[b,c,h,w] + y2[c]

Strategy: data-parallel over batch — 8 samples, 8 NeuronCores, one sample per
core; the tiny 1x1-path params are replicated. Per core x is [512, 16384] f32
(32 MiB) and the kernel is HBM-bound. HBM traffic is the whole game: x must
be consumed twice (pool, then broadcast add) but SBUF can hold the full
sample only at reduced precision. Pass 1 streams x in fp32, converts it to a
fully SBUF-resident bf16 copy (16 MiB) while accumulating the pool sums in
fp32; pass 2 adds y2 from the bf16 copy — no second read. Traffic is the
64 MiB floor (32 read + 32 write). bf16 rounding of x adds ~1.6e-3 relative
error against a 2e-2 budget.

Measured on this rig: pure-read streams ~313 GB/s with 1 MiB tiles; single
large transfers read at up to 425 GB/s, so loads use 4 MiB tiles ([128, 8192]
= 32 KiB per DMA descriptor) to cut per-transfer overheads. Stores run ~417
GB/s either way. The pool->y1->y2 barrier is minimized by tapering the last
chunk's tiles down to 512 columns, pre-reducing earlier partials, issuing
y1's accumulating matmuls per-chunk as sums complete, and running the whole
y-path in bf16 (fp32 matmuls cost 2 PE passes each; bf16 cost 1).

Host-side folding (all on tiny [C]-sized tensors):
    wg = (w_guide / HW).T          -> pool division folded into first matmul
    wf = (w_fuse * bn_scale).T     -> BN scale folded into second matmul
    b2 = beta - mean * bn_scale    -> BN shift applied as bias before relu6
All three are packed into one [128, 1152] tensor so the parameter load is a
single well-formed DMA (4.5 KiB lines) at the head of the ring.
"""

import numpy as np

from concourse import bass, mybir, tile
from concourse.bass_utils import run_bass_kernel_spmd

# Problem shapes (nn_GCF_FPGA_68032281969033), hardcoded per harness contract.
B, C, H, W = 8, 512, 128, 128
HW = H * W
R = 128
P = 128
BN_EPS = 1e-5

M_CHUNKS = C // P        # channel chunks of 128 partitions
PKW = 1152               # packed params: wg 512 | wf 512 | b2 128

# Load tile widths: big tiles for bandwidth, tapered tail on the last chunk
# so the pool barrier sees only a ~0.5 us conversion tail.
LOAD_FULL = [8192, 8192]
LOAD_TAIL = [8192, 4096, 2048, 1024, 512, 512]
# Store tile widths: tapered head so the first store issues right after y2.
STORE_HEAD = [512, 512, 1024, 2048, 4096, 8192]
STORE_FULL = [8192, 8192]

W_BUFS = 2               # [128, 8192] fp32 slots shared by pass 1 and pass 2

FP32 = mybir.dt.float32
BF16 = mybir.dt.bfloat16
AX = mybir.AxisListType.X
ALU = mybir.AluOpType
ACTF = mybir.ActivationFunctionType


def _scalar_share(w: int) -> int:
    # Balance ScalarE (one pass) vs DVE (copy + reduce) conversion halves.
    return min(w, max(32, (w * 2 // 3) & ~31))


def _build_program() -> bass.Bass:
    nc = bass.Bass()
    x_d = nc.declare_dram_parameter("x", [C, HW], FP32, isOutput=False)
    pk_d = nc.declare_dram_parameter("pk", [P, PKW], FP32, isOutput=False)
    out_d = nc.declare_dram_parameter("out", [C, HW], FP32, isOutput=True)

    with tile.TileContext(nc) as tc:
        with (
            tc.tile_pool(name="params", bufs=1) as ppool,
            tc.tile_pool(name="cache", bufs=1) as cpool,
            tc.tile_pool(name="work", bufs=W_BUFS) as wpool,
            tc.tile_pool(name="psum", bufs=1, space="PSUM") as qpool,
        ):
            pk_raw = ppool.tile([P, PKW], FP32, tag="pk_raw")
            nc.sync.dma_start(out=pk_raw[:], in_=pk_d[:])

            # Matmul (LDWEIGHTS) instructions only get one sync-wait slot in
            # walrus codegen; staging weights through DVE copies makes every
            # matmul input DVE-produced -> a single DVE wait. The copies also
            # cast to bf16 so each matmul is one PE pass instead of fp32's two.
            wg_b = ppool.tile([P, C], BF16, tag="wg_b")
            nc.vector.tensor_copy(out=wg_b[:], in_=pk_raw[:, 0:512])
            wf_b = ppool.tile([P, C], BF16, tag="wf_b")
            nc.vector.tensor_copy(out=wf_b[:], in_=pk_raw[:, 512:1024])
            b2_t = ppool.tile([P, M_CHUNKS], FP32, tag="b2")
            nc.vector.tensor_copy(out=b2_t[:], in_=pk_raw[:, 1024 : 1024 + M_CHUNKS])

            part_t = ppool.tile([P, 32], FP32, tag="part")
            sums_t = ppool.tile([P, M_CHUNKS], FP32, tag="sums")
            sums_b = ppool.tile([P, M_CHUNKS], BF16, tag="sums_b")
            y1_b = ppool.tile([P, 1], BF16, tag="y1")
            y2_t = ppool.tile([P, M_CHUNKS], FP32, tag="y2")

            cache = [cpool.tile([P, HW], BF16, tag=f"c{m}", name=f"c{m}")
                     for m in range(M_CHUNKS)]

            p1 = qpool.tile([P, 1], FP32, tag="p1")

            def conv_tile(t, w, m, off, col):
                """Convert landing tile t[:, :w] into cache[m][:, off:off+w]
                (bf16) while row-summing into part_t cols col, col+1.
                ScalarE does the first share in one fused pass; DVE does the
                rest as copy + reduce. Returns number of partial cols (2)."""
                ws = _scalar_share(w)
                nc.scalar.activation(
                    out=cache[m][:, off : off + ws], in_=t[:, :ws],
                    func=ACTF.Copy, accum_out=part_t[:, col : col + 1],
                )
                if ws < w:
                    dst = cache[m][:, off + ws : off + w]
                    nc.vector.tensor_copy(out=dst, in_=t[:, ws:w])
                    nc.vector.reduce_sum(
                        out=part_t[:, col + 1 : col + 2], in_=dst, axis=AX
                    )
                else:
                    nc.vector.memset(part_t[:, col + 1 : col + 2], 0.0)

            # Pass 1: stream x, convert to resident bf16, accumulate pool sums.
            # Partial-col layout: chunks 0-2 use cols 2*t per tile; chunk 3's
            # widths through 1024 pre-reduce into col PRE so the final sum
            # after the last 512-wide tile reduces only 5 contiguous cols.
            pcol = 0
            for m in range(M_CHUNKS):
                widths = LOAD_FULL if m < M_CHUNKS - 1 else LOAD_TAIL
                lo = pcol
                off = 0
                for j, w in enumerate(widths):
                    t = wpool.tile([P, 8192], FP32, tag="w", name="t")
                    nc.sync.dma_start(
                        out=t[:, :w], in_=x_d[m * P : (m + 1) * P, off : off + w]
                    )
                    if m == M_CHUNKS - 1 and j == len(widths) - 2:
                        # pre-reduce earlier partials while the tail flies
                        nc.vector.reduce_sum(
                            out=part_t[:, pcol : pcol + 1],
                            in_=part_t[:, lo:pcol], axis=AX,
                        )
                        lo = pcol
                        pcol += 1
                    conv_tile(t, w, m, off, pcol)
                    off += w
                    pcol += 2
                nc.vector.reduce_sum(
                    out=sums_t[:, m : m + 1], in_=part_t[:, lo:pcol], axis=AX
                )
                nc.vector.tensor_copy(
                    out=sums_b[:, m : m + 1], in_=sums_t[:, m : m + 1]
                )
                nc.tensor.matmul(
                    p1[:],
                    wg_b[:, m * P : (m + 1) * P],
                    sums_b[:, m : m + 1],
                    start=(m == 0),
                    stop=(m == M_CHUNKS - 1),
                )

            # y1 = relu6(p1); y2 = relu6(wf.T @ y1 + b2).
            nc.vector.tensor_scalar(
                out=y1_b[:], in0=p1[:], scalar1=0.0, scalar2=6.0,
                op0=ALU.max, op1=ALU.min,
            )
            p2 = qpool.tile([P, M_CHUNKS], FP32, tag="p2")
            for m in range(M_CHUNKS):
                nc.tensor.matmul(
                    p2[:, m : m + 1],
                    wf_b[:, m * P : (m + 1) * P],
                    y1_b[:],
                    start=True,
                    stop=True,
                )
            nc.vector.tensor_add(out=y2_t[:], in0=p2[:], in1=b2_t[:])
            nc.vector.tensor_scalar(
                out=y2_t[:], in0=y2_t[:], scalar1=0.0, scalar2=6.0,
                op0=ALU.max, op1=ALU.min,
            )

            # Pass 2: out = bf16(x) + y2[channel], straight from SBUF.
            # Tiles alternate ScalarE/DVE so the add stream runs ~3x ahead of
            # the store DMAs; the tapered head starts the store stream as soon
            # after y2 as possible.
            k = 0
            for m in range(M_CHUNKS):
                widths = STORE_HEAD if m == 0 else STORE_FULL
                off = 0
                for w in widths:
                    s = wpool.tile([P, 8192], FP32, tag="w", name="s")
                    src = cache[m][:, off : off + w]
                    if k % 2 == 0:
                        nc.scalar.add(out=s[:, :w], in_=src, add=y2_t[:, m : m + 1])
                    else:
                        nc.vector.tensor_scalar_add(
                            out=s[:, :w], in0=src, scalar1=y2_t[:, m : m + 1]
                        )
                    nc.sync.dma_start(
                        out=out_d[m * P : (m + 1) * P, off : off + w], in_=s[:, :w]
                    )
                    off += w
                    k += 1

    _hoist_excess_waits(nc)
    return nc


# walrus codegen has per-instruction sync-wait slot limits (the Matmult
# LDWEIGHTS struct fits one wait; the DMA DIRECT2D struct fits two). Tile's
# sem assignment is not transitively minimal and can exceed them. Excess waits
# are hoisted into standalone EventSemaphore instructions placed right before
# the instruction on the same engine queue — identical semantics (inline DMA
# waits execute at the issuing sequencer too), just a different encoding.
_WAIT_CAPS = {
    "InstMatmult": 1,
    "InstActivation": 1,
    "InstDMACopy": 1,
    "InstTensorReduce": 1,
    "InstTensorScalarPtr": 1,
    "InstTensorTensor": 1,
    "InstTensorCopy": 1,
    "InstMemset": 1,
    "InstDrain": 1,
}


def _hoist_excess_waits(nc: bass.Bass) -> None:
    n = 0
    for bb in nc.main_func.blocks:
        il = bb.instructions
        new_list = []
        for ins in il:
            si = ins.sync_info
            cap = _WAIT_CAPS.get(type(ins).__name__)
            if si is not None and cap is not None and len(si.on_wait) > cap:
                waits = list(si.on_wait)
                for w in waits[cap:]:
                    n += 1
                    es = mybir.InstEventSemaphore(
                        name=f"I-hoistwait-{n}",
                        engine=ins.engine,
                        sync_info=mybir.SyncInfo(on_wait=[w], on_update=[]),
                    )
                    new_list.append(es)
                ins.sync_info = mybir.SyncInfo(
                    on_wait=waits[:cap], on_update=list(si.on_update)
                )
            new_list.append(ins)
        if len(new_list) != len(il):
            il[:] = new_list


_NC = None


def _get_nc() -> bass.Bass:
    global _NC
    if _NC is None:
        _NC = _build_program()
    return _NC


def _prep_in_maps(x, w_guide, w_fuse, bn_gamma, bn_beta, bn_mean, bn_var):
    x = np.asarray(x, dtype=np.float32)
    w_guide = np.asarray(w_guide, dtype=np.float32)
    w_fuse = np.asarray(w_fuse, dtype=np.float32)
    bn_gamma = np.asarray(bn_gamma, dtype=np.float32)
    bn_beta = np.asarray(bn_beta, dtype=np.float32)
    bn_mean = np.asarray(bn_mean, dtype=np.float32)
    bn_var = np.asarray(bn_var, dtype=np.float32)

    scale = bn_gamma / np.sqrt(bn_var + np.float32(BN_EPS))
    wg = (w_guide / np.float32(HW)).T            # [C, R]
    wf = (w_fuse * scale[:, None]).T             # [R, C]
    b2 = (bn_beta - bn_mean * scale).reshape(M_CHUNKS, P).T  # [P, 4]

    pk = np.zeros((P, PKW), dtype=np.float32)
    # wg packed as [p, m*128 + r] = wg[m*128 + p, r]
    pk[:, 0:512] = wg.reshape(M_CHUNKS, P, R).transpose(1, 0, 2).reshape(P, 512)
    pk[:, 512:1024] = wf
    pk[:, 1024 : 1024 + M_CHUNKS] = b2

    xs = np.ascontiguousarray(x.reshape(B, C, HW))
    return [{"x": xs[i], "pk": pk} for i in range(B)]


def run(inputs: dict, **kwargs):
    """Run the SPMD kernel; returns the BassKernelResults (for profiling)."""
    nc = _get_nc()
    in_maps = _prep_in_maps(**inputs)
    return run_bass_kernel_spmd(nc, in_maps, core_ids=list(range(B)), **kwargs)


def kernel(**inputs) -> np.ndarray:
    res = run(inputs)
    out = np.stack([np.asarray(res.results[i]["out"]) for i in range(B)], axis=0)
    return out.reshape(B, C, H, W).astype(np.float32, copy=False)
